# revision 45
# baseline (speedup 1.0000x reference)
"""CRNN Trainium2 kernel: patchify-conv -> 3x3 conv -> pool -> GRU encoder ->
autoregressive GRU decoder. Pure data-parallel over batch (32 -> 8 cores x 4).

v2: fp8-e4m3 DoubleRow convs (2x PE rate, 4x less frame DMA), conv2 merged to
one N=512 matmul stream, biases folded into PSUM via K=1/K=10 matmuls, dual
HW DMA queues (Sync for weights, Activation for frames), persistent
gutter-layout conv2 input (zeroed once), shortened GRU cell dependency chain.
"""

import os
import sys

for _p in ("/opt/trn_rl_repo", "/root/.axon_site/_ro/trn_rl_repo"):
    if os.path.isdir(_p) and _p not in sys.path:
        sys.path.insert(0, _p)

import numpy as np

import concourse.bass as bass  # noqa: E402
import concourse.mybir as mybir  # noqa: E402
import concourse.tile as tile  # noqa: E402
from concourse import bacc  # noqa: E402
from concourse.bass_utils import run_bass_kernel_spmd  # noqa: E402

F32 = mybir.dt.float32
F8 = mybir.dt.float8e4
AF = mybir.ActivationFunctionType
ALU = mybir.AluOpType
DR = mybir.MatmulPerfMode.DoubleRow

# Model dims (hardcoded from the problem spec)
B, L, DS, DA, DC, DRN, DO, HOR = 32, 16, 12, 16, 64, 256, 2, 10
NCORES, BPC = 8, 4          # batch per core
NG, FPG = 8, 8              # 8 groups of 8 frames per core (frame idx = l*4+b)
BN_EPS = 1e-5
S1, S2 = 8.0, 16.0          # fp8 weight scales (conv1, conv2)

MM_DT_RNN = os.environ.get("BASS_MM_DT_RNN", "f16")


def _dt_of(tag):
    return {"f32": mybir.dt.float32, "f32r": mybir.dt.float32r,
            "bf16": mybir.dt.bfloat16, "f16": mybir.dt.float16}[tag]

LAST_EXEC_NS = None
LAST_RESULTS = None


def _layout(entries):
    """entries: (name, rows, width[, row0]) -> dict + total cols."""
    out = {}
    cols = 0
    for e in entries:
        name, rows, width = e[0], e[1], e[2]
        row0 = e[3] if len(e) > 3 else 0
        out[name] = (row0, rows, cols, width)
        cols += width
    return out, cols


# matmul operands (RNN matmul dtype)
SMM_LAYOUT, SMM_COLS = _layout([
    ("xt", 12, 64),          # per-core x transposed, col = l*4+b
    ("a0t", 12, 16),
    ("ait", 16, 16),
    ("anT", 80, 256),        # [an_w[:,16:80].T ; an_w[:,0:16].T] rows
    # decoder bias-into-psum operands: one matmul per psum tile
    # (lhsT = bias chunks as rows, rhs = chunk->column selector)
    ("brz4", 4, 128),        # (b_ih+b_hh) rz chunks
    ("bin2", 2, 128),        # b_ih n chunks
    ("bhhn2", 2, 128),       # b_hh n chunks
    ("fib2", 2, 128),        # fi_b chunks
    ("bsgi6", 6, 128),       # GI bias chunks
    ("sel16", 4, 16),        # selector: col n lights chunk n//4
    ("sel8", 2, 8),
    ("sel48", 6, 48),        # selector: col n lights chunk n//8
])
# bias/affine tables (always fp32)
SMB_LAYOUT, SMB_COLS = _layout([
    ("b2t", 64, 392),        # conv2 bias table (64, 49) tiled x8, x(S1*S2)
    ("pscale", 64, 1),       # inv/49/(S1*S2)
    ("pshift", 64, 1),
    ("a0b", 16, 1),
    ("aib", 16, 1),
    ("anb", 128, 2),         # an_b chunks as cols
    ("bsgi", 128, 6),        # b_ih + b_hh (rz) / b_ih (n), chunk cols
    ("bhhn", 128, 8),        # b_hh n-part, tiled x4 (encoder cell)
    ("fnb", 2, 1),
])

SHIFTS = [(dh, dw) for dh in range(3) for dw in range(3)]


def build_nc():
    nc = bacc.Bacc("TRN2", target_bir_lowering=False, debug=False,
                   num_devices=NCORES)
    mm_rnn = _dt_of(MM_DT_RNN)
    MR = mm_rnn

    h_fr = nc.dram_tensor("fr", [NG, 128, 6 * 392], F8, kind="ExternalInput")
    h_smm = nc.dram_tensor("smm", [128, SMM_COLS], MR, kind="ExternalInput")
    h_smb = nc.dram_tensor("smb", [128, SMB_COLS], F32, kind="ExternalInput")
    h_w1 = nc.dram_tensor("w1", [128, 6 * 576], F8, kind="ExternalInput")
    h_w2 = nc.dram_tensor("w2", [128, 45 * 64], F8, kind="ExternalInput")
    h_wih = nc.dram_tensor("wih", [128, 2 * 768], MR, kind="ExternalInput")
    h_whh = nc.dram_tensor("whh", [128, 2 * 768], MR, kind="ExternalInput")
    h_fi = nc.dram_tensor("fiw", [128, 2 * 256], MR, kind="ExternalInput")
    h_fn = nc.dram_tensor("fnw", [128, 4], MR, kind="ExternalInput")
    h_out = nc.dram_tensor("out", [2, 4 * HOR], F32, kind="ExternalOutput")

    def mm(out, lhsT, rhs, **kw):
        nc.tensor.matmul(out, lhsT, rhs, skip_group_check=True, **kw)

    with tile.TileContext(nc) as tc:
        from contextlib import ExitStack
        with ExitStack() as ctx:
            cpool = ctx.enter_context(tc.tile_pool(name="const", bufs=1))
            xin_pool = ctx.enter_context(tc.tile_pool(name="xin", bufs=3))
            work = ctx.enter_context(tc.tile_pool(name="work", bufs=4))
            state = ctx.enter_context(tc.tile_pool(name="state", bufs=1))
            hpool = ctx.enter_context(tc.tile_pool(name="h", bufs=3))
            ps1 = ctx.enter_context(
                tc.tile_pool(name="ps1", bufs=3, space="PSUM"))
            ps2 = ctx.enter_context(
                tc.tile_pool(name="ps2", bufs=2, space="PSUM"))
            psr = ctx.enter_context(
                tc.tile_pool(name="psr", bufs=3, space="PSUM"))

            # ---- constants to SBUF ----
            # qSync order = first-use order (conv1 g0 only needs w1;
            # adapters/an need smm/smb later); frames + w2 ride the
            # Activation HWDGE queue in parallel.
            # w1 split per K-pair so conv1's first matmul starts as soon as
            # pair 0 lands instead of waiting for the full tensor
            w1 = cpool.tile([128, 6, 576], F8, tag="w1")
            for j in range(3):
                nc.sync.dma_start(
                    w1[:, 2 * j:2 * j + 2, :].rearrange("p a b -> p (a b)"),
                    h_w1[:, 2 * j * 576:(2 * j + 2) * 576])
            smb = cpool.tile([128, SMB_COLS], F32, tag="smb")
            nc.sync.dma_start(smb[:], h_smb[:])
            smm = cpool.tile([128, SMM_COLS], MR, tag="smm")
            nc.sync.dma_start(smm[:], h_smm[:])
            w2 = cpool.tile([128, 45, 64], F8, tag="w2")
            nc.scalar.dma_start(w2[:].rearrange("p a b -> p (a b)"), h_w2[:])
            wih = cpool.tile([128, 2 * 768], MR, tag="wih")
            nc.sync.dma_start(wih[:], h_wih[:])
            whh = cpool.tile([128, 2 * 768], MR, tag="whh")
            nc.sync.dma_start(whh[:], h_whh[:])
            fiw = cpool.tile([128, 2 * 256], MR, tag="fiw")
            nc.sync.dma_start(fiw[:], h_fi[:])
            fnw = cpool.tile([128, 4], MR, tag="fnw")
            nc.sync.dma_start(fnw[:], h_fn[:])

            def sv(name):  # matmul-operand view (RNN dtype)
                r0, rows, off, width = SMM_LAYOUT[name]
                return smm[r0:r0 + rows, off:off + width]

            def svc(name, c0, w):
                r0, rows, off, width = SMM_LAYOUT[name]
                assert c0 + w <= width
                return smm[r0:r0 + rows, off + c0:off + c0 + w]

            def svf(name):  # fp32 bias/affine view
                r0, rows, off, width = SMB_LAYOUT[name]
                return smb[r0:r0 + rows, off:off + width]

            def svcf(name, c0, w):
                r0, rows, off, width = SMB_LAYOUT[name]
                assert c0 + w <= width
                return smb[r0:r0 + rows, off + c0:off + c0 + w]

            # PE warm-up: the tensor engine p-state ramps only under load,
            # and the first real matmuls otherwise run ~2.5x slow. Burn the
            # DMA-wait window (~2.5-10us) with throwaway matmuls on a
            # zeroed tile.
            # (wtile is read uninitialized — the product is never consumed,
            # so garbage values are harmless, and skipping the memset lets
            # the warm-up start with zero dependencies)
            wtile = work.tile([128, 512], F8, tag="warm")
            pw = psr.tile([64, 512], F32, tag="ps")
            for wi in range(20):
                mm(pw[:], wtile[:, 0:64], wtile[:, 0:512],
                   start=True, stop=True)

            # ---- persistent state tiles ----
            # conv2 gutter-flat layout, double-buffered across groups: per
            # frame an 8x8 cell grid (row 0 / col 0 zero gutters); LEAD/TAIL
            # pads absorb shift-window spill. Chunks 0-4 hold conv1 output;
            # chunks 5-7 hold shifted copies of chunk 4 (shift +1 / raw /
            # shift +8) so pairs of 3x3 taps on the half-size last ic chunk
            # contract in single DoubleRow matmuls. Zeroed once; only data
            # cells are rewritten per group.
            LEAD, FW = 16, 16 + 8 * 64 + 16  # 544 per ic-chunk
            f1s = state.tile([128, 2, 8, FW], F8, tag="f1s")
            nc.gpsimd.memset(f1s[:], 0.0)
            s2 = state.tile([16, 64], MR, tag="s2")
            s_enc = state.tile([128, 2, 64], MR, tag="senc")
            GI = state.tile([128, 6, 64], F32, tag="gi")
            preds = state.tile([2, 4 * HOR], F32, tag="preds")

            # ---- state adapters: s1 = relu(a0 x); s2 = s1 + relu(ai s1) ----
            # Emitted mid-group-0 so the conv1 matmuls (which only need
            # w1+xin0) lead the tensor queue instead of stalling on smm.
            def emit_adapters():
                pa = psr.tile([16, 64], F32, tag="ps")
                mm(pa[:], sv("a0t"), sv("xt"), start=True, stop=True)
                s1 = work.tile([16, 64], MR, tag="s1")
                nc.scalar.activation(s1[:], pa[:], AF.Relu, bias=svf("a0b"))
                pb = psr.tile([16, 64], F32, tag="ps")
                mm(pb[:], sv("ait"), s1[:], start=True, stop=True)
                s1b = work.tile([16, 64], MR, tag="s1")
                nc.scalar.activation(s1b[:], pb[:], AF.Relu, bias=svf("aib"))
                nc.vector.tensor_add(s2[:], s1[:], s1b[:])

            # encoder hidden state
            h_cur = hpool.tile([128, 8], MR, tag="h")
            nc.gpsimd.memset(h_cur[:], 0.0)

            def gru_tail(rz, gi_n, gh_n, h_prev, gi_3d=False):
                """Common GRU cell tail after sigmoid: returns h_new.
                Chain: a2 -> a3 -> tanh -> q -> hn, with (1-z) and z*h
                computed on GpSimd off the critical path."""
                omz = work.tile([128, 8], F32, tag="omz")
                nc.gpsimd.tensor_scalar(omz[:], rz[:, 8:16], -1.0, 1.0,
                                        op0=ALU.mult, op1=ALU.add)
                zh = work.tile([128, 8], F32, tag="zh")
                nc.gpsimd.tensor_mul(zh[:], rz[:, 8:16], h_prev[:])
                a2 = work.tile([128, 8], F32, tag="g8b")
                nc.vector.tensor_mul(a2[:], rz[:, 0:8], gh_n)
                a3 = work.tile([128, 8], F32, tag="g8c")
                if gi_3d:
                    nc.vector.tensor_add(
                        a3[:].rearrange("p (c b) -> p c b", b=4),
                        a2[:].rearrange("p (c b) -> p c b", b=4), gi_n)
                else:
                    nc.vector.tensor_add(a3[:], a2[:], gi_n)
                nt = work.tile([128, 8], F32, tag="g8d")
                nc.scalar.activation(nt[:], a3[:], AF.Tanh)
                q = work.tile([128, 8], F32, tag="g8e")
                nc.vector.tensor_mul(q[:], omz[:], nt[:])
                h_new = hpool.tile([128, 8], MR, tag="h")
                nc.vector.tensor_add(h_new[:], q[:], zh[:])
                return h_new

            def enc_step(t, h_prev):
                # split psum tiles: the rz chain must not wait on n-gate mm
                prz = psr.tile([128, 16], F32, tag="ps")
                for mc in range(4):
                    reg = prz[:, mc * 4:(mc + 1) * 4]
                    for kc in range(2):
                        mm(reg,
                           whh[:, kc * 768 + mc * 128:kc * 768 + (mc + 1) * 128],
                           h_prev[:, kc * 4:(kc + 1) * 4],
                           start=(kc == 0), stop=(kc == 1))
                pn = psr.tile([128, 8], F32, tag="ps")
                for mc2 in range(2):
                    reg = pn[:, mc2 * 4:(mc2 + 1) * 4]
                    for kc in range(2):
                        mm(reg, whh[:, kc * 768 + (4 + mc2) * 128:
                                    kc * 768 + (5 + mc2) * 128],
                           h_prev[:, kc * 4:(kc + 1) * 4],
                           start=(kc == 0), stop=(kc == 1))
                gi_rz = GI[:, 0:4, t * 4:(t + 1) * 4]
                gi_n = GI[:, 4:6, t * 4:(t + 1) * 4]
                pre = work.tile([128, 16], F32, tag="g16")
                nc.vector.tensor_add(
                    pre[:].rearrange("p (c b) -> p c b", b=4), gi_rz,
                    prz[:].rearrange("p (c b) -> p c b", b=4))
                rz = work.tile([128, 16], F32, tag="g16b")
                nc.scalar.activation(rz[:], pre[:], AF.Sigmoid)
                a1 = work.tile([128, 8], F32, tag="g8")
                nc.vector.tensor_add(a1[:], pn[:], svf("bhhn"))
                return gru_tail(rz, gi_n, a1[:], h_prev, gi_3d=True)

            # ---- conv + features + GI, per group of 8 frames ----
            for g in range(NG):
                xin = xin_pool.tile([128, 6, 392], F8, tag="xin")
                nc.scalar.dma_start(
                    xin[:].rearrange("p a b -> p (a b)"), h_fr[g])

                # conv1 (DoubleRow, 3 K-pairs x 5 M-chunks) -> f1 data
                # cells; m=4 first so its shifted copies (below) finish
                # while the other chunks still stream
                for m in (4, 0, 1, 2, 3):
                    msz = 128 if m < 4 else 64
                    p1 = ps1.tile([msz, 392], F32, tag="c1")
                    for j in range(3):
                        mm(p1[:], w1[:, 2 * j:2 * j + 2, m * 128:m * 128 + msz],
                           xin[:, 2 * j:2 * j + 2, :],
                           start=(j == 0), stop=(j == 2), perf_mode=DR)
                    dst = f1s[0:msz, g % 2, m, LEAD:LEAD + 512].rearrange(
                        "p (f a b) -> p f a b", a=8, b=8)[:, :, 1:8, 1:8]
                    src = p1[:].rearrange("p (f a b) -> p f a b", a=7, b=7)
                    bsel = g % 2
                    if m in (0, 1, 2):
                        nc.vector.tensor_copy(dst, src)
                    else:
                        nc.scalar.activation(dst, src, AF.Copy)
                    if m == 4:
                        # shifted copies of chunk 4 for tap pairing
                        nc.gpsimd.tensor_copy(f1s[0:64, bsel, 5, 0:FW - 1],
                                              f1s[0:64, bsel, 4, 1:FW])
                        nc.gpsimd.tensor_copy(f1s[0:64, bsel, 6, :],
                                              f1s[0:64, bsel, 4, :])
                        nc.vector.tensor_copy(f1s[0:64, bsel, 7, 0:FW - 8],
                                              f1s[0:64, bsel, 4, 8:FW])

                # conv2, all 8 frames, dead b=0 gutter columns skipped via
                # strided APs (N=448). Chunks 0-3: 9 taps x 2 DoubleRow
                # K-pairs. Chunk 4 (64 rows): taps paired through the
                # shifted copies: (s0,s1),(s3,s4),(s6,s7) via chunks (4,5),
                # (s2,s5) via (6,7), s8 single.
                p2 = ps2.tile([64, 512], F32, tag="c2")
                out2 = p2[:].rearrange("p (fa b) -> p fa b", b=8)[:, :, 1:8]

                def rhs4(c0, nch, a):
                    v = f1s[:, bsel, c0:c0 + nch, a:a + 512].rearrange(
                        "p k (fa b) -> p k fa b", b=8)[:, :, :, 0:7]
                    return v

                for si, (dh, dw) in enumerate(SHIFTS):
                    a = LEAD + (dh - 1) * 8 + (dw - 1) + 1
                    for j in range(2):
                        mm(out2, w2[:, (si * 4 + 2 * j):(si * 4 + 2 * j + 2), :],
                           rhs4(2 * j, 2, a),
                           start=(si == 0 and j == 0), stop=False,
                           perf_mode=DR)
                for bi, (slot, c0, dlt) in enumerate(
                        [(36, 4, -9), (38, 4, -1), (40, 4, 7), (42, 6, -7)]):
                    mm(out2, w2[:, slot:slot + 2, :],
                       rhs4(c0, 2, LEAD + dlt + 1),
                       start=False, stop=False, perf_mode=DR)
                mm(out2, w2[:, 44, :],
                   f1s[:, bsel, 4, LEAD + 10:LEAD + 10 + 512].rearrange(
                       "p (fa b) -> p fa b", b=8)[:, :, 0:7],
                   start=False, stop=True)

                if g == 0:
                    emit_adapters()

                # epilogue: +bias (vector) -> relu (scalar) -> sum -> affine
                pv = p2[:].rearrange("p (f a b) -> p f a b",
                                     a=8, b=8)[:, :, 1:8, 1:8]
                b2v = svf("b2t").rearrange("p (f a b) -> p f a b", a=7, b=7)
                t0 = work.tile([64, 8, 7, 7], F32, tag="ep0")
                nc.vector.tensor_add(t0[:], pv, b2v)
                t1 = work.tile([64, 8, 7, 7], F32, tag="ep")
                nc.scalar.activation(t1[:], t0[:], AF.Relu)
                red = work.tile([64, 8], F32, tag="red")
                nc.vector.tensor_reduce(red[:], t1[:],
                                        axis=mybir.AxisListType.XY,
                                        op=ALU.add)
                feats = work.tile([80, 8], MR, tag="feats")
                nc.scalar.activation(feats[0:64, :], red[:], AF.Identity,
                                     bias=svf("pshift"), scale=svf("pscale"))

                # an: relu(an_w [s2; feats] + an_b), one K=80 matmul per half
                gcol = slice(g * FPG, (g + 1) * FPG)
                nc.gpsimd.tensor_copy(feats[64:80, :], s2[:, gcol])
                for mc in range(2):
                    pan = psr.tile([128, FPG], F32, tag="ps")
                    mm(pan[:], svc("anT", mc * 128, 128), feats[:],
                       start=True, stop=True)
                    nc.scalar.activation(s_enc[:, mc, gcol], pan[:], AF.Relu,
                                         bias=svcf("anb", mc, 1))

                # GI = w_ih @ s_enc + (b_ih + b_hh fold) for these 8 cols:
                # one psum tile, bias via selector matmul, single ACT out
                pgi = psr.tile([128, 48], F32, tag="ps")
                mm(pgi[:], sv("bsgi6"), sv("sel48"), start=True, stop=False)
                for mc in range(6):
                    for kc in range(2):
                        mm(pgi[:, mc * 8:(mc + 1) * 8],
                           wih[:, kc * 768 + mc * 128:kc * 768 + (mc + 1) * 128],
                           s_enc[:, kc, gcol],
                           start=False, stop=(mc == 5 and kc == 1))
                nc.scalar.activation(
                    GI[:, :, gcol],
                    pgi[:].rearrange("p (c b) -> p c b", b=8), AF.Identity)

                # encoder steps that become ready after this group
                h_cur = enc_step(2 * g, h_cur)
                h_cur = enc_step(2 * g + 1, h_cur)

            # ---- decoder ----
            # Separate psum tiles per gate group so each consumer waits only
            # on its own matmuls (psum deps are tile-granular); biases enter
            # psum via selector matmuls so the elementwise chain reads psum
            # raw. Emission order puts everything that depends only on
            # hn(t-1) (whh parts) ahead of the xr(t-1)-dependent work, and
            # defers fn(t-1) behind the whh block, so the tensor queue keeps
            # moving during the fi/xr window.
            def emit_fn(x, tt):
                pfn = psr.tile([2, 4], F32, tag="ps")
                for kc in range(2):
                    mm(pfn[:], fnw[:, kc * 2:(kc + 1) * 2],
                       x[:, kc * 4:(kc + 1) * 4],
                       start=(kc == 0), stop=(kc == 1))
                nc.scalar.activation(preds[:, tt * 4:(tt + 1) * 4], pfn[:],
                                     AF.Tanh, bias=svf("fnb"))

            xi, hh = h_cur, h_cur
            for t in range(HOR):
                prz = psr.tile([128, 16], F32, tag="ps")
                mm(prz[:], sv("brz4"), sv("sel16"), start=True, stop=False)
                for mc in range(4):
                    reg = prz[:, mc * 4:(mc + 1) * 4]
                    for kc in range(2):
                        mm(reg, whh[:, kc * 768 + mc * 128:
                                    kc * 768 + (mc + 1) * 128],
                           hh[:, kc * 4:(kc + 1) * 4],
                           start=False, stop=False)
                pghn = psr.tile([128, 8], F32, tag="ps")
                mm(pghn[:], sv("bhhn2"), sv("sel8"), start=True, stop=False)
                for mc2 in range(2):
                    reg = pghn[:, mc2 * 4:(mc2 + 1) * 4]
                    for kc in range(2):
                        mm(reg, whh[:, kc * 768 + (4 + mc2) * 128:
                                    kc * 768 + (5 + mc2) * 128],
                           hh[:, kc * 4:(kc + 1) * 4],
                           start=False, stop=(mc2 == 1 and kc == 1))
                # allocate pgin now (slot s2) so it never inherits prz's
                # ring slot; its wih matmuls are emitted below
                pgin = psr.tile([128, 8], F32, tag="ps")
                mm(pgin[:], sv("bin2"), sv("sel8"), start=True, stop=False)
                if t > 0:
                    emit_fn(xi, t - 1)
                for mc in range(4):
                    reg = prz[:, mc * 4:(mc + 1) * 4]
                    for kc in range(2):
                        mm(reg, wih[:, kc * 768 + mc * 128:
                                    kc * 768 + (mc + 1) * 128],
                           xi[:, kc * 4:(kc + 1) * 4],
                           start=False, stop=(mc == 3 and kc == 1))
                for mc2 in range(2):
                    reg = pgin[:, mc2 * 4:(mc2 + 1) * 4]
                    for kc in range(2):
                        mm(reg, wih[:, kc * 768 + (4 + mc2) * 128:
                                    kc * 768 + (5 + mc2) * 128],
                           xi[:, kc * 4:(kc + 1) * 4],
                           start=False, stop=(mc2 == 1 and kc == 1))
                # fi bias rides the idle tensor window before hn is ready
                pfi = psr.tile([128, 8], F32, tag="ps")
                mm(pfi[:], sv("fib2"), sv("sel8"), start=True, stop=False)

                rz = work.tile([128, 16], F32, tag="g16b")
                nc.scalar.activation(rz[:], prz[:], AF.Sigmoid)
                hn = gru_tail(rz, pgin[:], pghn[:], hh)

                # final_i residual: xr = hn + relu(fi hn + fi_b)
                for mc2 in range(2):
                    reg = pfi[:, mc2 * 4:(mc2 + 1) * 4]
                    for kc2 in range(2):
                        mm(reg, fiw[:, kc2 * 256 + mc2 * 128:
                                    kc2 * 256 + (mc2 + 1) * 128],
                           hn[:, kc2 * 4:(kc2 + 1) * 4],
                           start=False, stop=(mc2 == 1 and kc2 == 1))
                xr = hpool.tile([128, 8], MR, tag="xr")
                nc.vector.scalar_tensor_tensor(
                    xr[:], pfi[:], 0.0, hn[:],
                    op0=ALU.max, op1=ALU.add)
                xi, hh = xr, hn
            emit_fn(xi, HOR - 1)

            nc.sync.dma_start(h_out[:], preds[:])

    nc.finalize()
    return nc


# ---------------- host-side data prep ----------------

def _prep_frames(frames):
    """frames (32,16,3,112,112) -> per-core [NG, 128, 2352] patch-T fp8."""
    out = np.empty((NCORES, NG, 128, 6 * 392), mybir.dt.np(F8))
    fr = np.ascontiguousarray(frames, np.float32)
    for c in range(NCORES):
        fb = fr[c * BPC:(c + 1) * BPC]  # (4, 16, 3, 112, 112)
        a = fb.reshape(BPC, L, 3, 7, 16, 7, 16)
        # -> [l, b, ch, kh, kw, ph, pw]
        a = a.transpose(1, 0, 2, 4, 6, 3, 5)
        a = a.reshape(L, BPC, 768, 49)
        a = a.reshape(NG, 2, BPC, 6, 128, 49)
        # -> [g, k, p, li, b, s]
        a = a.transpose(0, 3, 4, 1, 2, 5)
        a = a.reshape(NG, 6, 128, 392)
        a = a.transpose(0, 2, 1, 3)  # [g, p, k, 392]
        out[c] = a.reshape(NG, 128, 6 * 392).astype(mybir.dt.np(F8))
    return out


def _prep_weights(iv):
    w = {}
    f8 = mybir.dt.np(F8)
    W1f = iv["cnn_w"].reshape(576, 768).astype(np.float32) * S1
    w["w1"] = np.ascontiguousarray(
        W1f.T.reshape(6, 128, 576).transpose(1, 0, 2).reshape(
            128, 6 * 576)).astype(f8)

    # conv2 weights, 45 lhsT blocks of [128, 64]:
    # slots 0-35: shift-major, (j, i) -> ic chunk 2j+i, for the chunk 0-3
    # DoubleRow pairs; slots 36-44: chunk-4 tap blocks in pairing order
    # (s0,s1),(s3,s4),(s6,s7),(s2,s5),(s8) matching the shifted-copy scheme.
    w2h = np.zeros((45, 128, 64), np.float32)
    T9 = []
    for dh in range(3):
        for dw in range(3):
            T9.append(iv["cnn1_w"][:, :, dh, dw].T.astype(np.float32) * S2)
    for s in range(9):
        for ch in range(4):
            w2h[s * 4 + ch] = T9[s][ch * 128:(ch + 1) * 128]
    for idx, s in enumerate([0, 1, 3, 4, 6, 7, 2, 5, 8]):
        w2h[36 + idx, 0:64] = T9[s][512:576]
    w["w2"] = np.ascontiguousarray(
        w2h.transpose(1, 0, 2).reshape(128, 45 * 64)).astype(f8)

    rdt = mybir.dt.np(_dt_of(MM_DT_RNN))
    for name, key in (("wih", "w_ih"), ("whh", "w_hh")):
        T = iv[key].T.astype(np.float32)  # (256, 768)
        w[name] = np.ascontiguousarray(
            T.reshape(2, 128, 768).transpose(1, 0, 2).reshape(
                128, 1536)).astype(rdt)
    T = iv["fi_w"].T.astype(np.float32)  # (256, 256)
    w["fiw"] = np.ascontiguousarray(
        T.reshape(2, 128, 256).transpose(1, 0, 2).reshape(128, 512)).astype(rdt)
    T = iv["fn_w"].T.astype(np.float32)  # (256, 2)
    w["fnw"] = np.ascontiguousarray(
        T.reshape(2, 128, 2).transpose(1, 0, 2).reshape(128, 4)).astype(rdt)
    return w


def _prep_smalls(iv, x, core):
    smm = np.zeros((128, SMM_COLS), mybir.dt.np(_dt_of(MM_DT_RNN)))
    smb = np.zeros((128, SMB_COLS), np.float32)

    def put(name, arr):
        if name in SMM_LAYOUT:
            r0, rows, off, width = SMM_LAYOUT[name]
            dst = smm
        else:
            r0, rows, off, width = SMB_LAYOUT[name]
            dst = smb
        a = np.asarray(arr, np.float32).reshape(rows, width)
        dst[r0:r0 + rows, off:off + width] = a.astype(dst.dtype)

    # conv2 position-dependent bias fold (conv1 bias + cnn1_b), x(S1*S2)
    # to match the psum scale
    M = np.einsum("oiab,i->oab", iv["cnn1_w"], iv["cnn_b"]).astype(np.float32)
    B2 = np.zeros((64, 7, 7), np.float32)
    for ph in range(7):
        for pw in range(7):
            acc = iv["cnn1_b"].astype(np.float32).copy()
            for dh in range(3):
                for dw in range(3):
                    if 0 <= ph + dh - 1 <= 6 and 0 <= pw + dw - 1 <= 6:
                        acc = acc + M[:, dh, dw]
            B2[:, ph, pw] = acc
    put("b2t", np.tile(B2.reshape(64, 49) * S1 * S2, (1, FPG)))

    inv = iv["bn_g"] / np.sqrt(iv["bn_v"] + BN_EPS)
    put("pscale", (inv / 49.0 / (S1 * S2))[:, None])
    put("pshift", (iv["bn_b"] - iv["bn_m"] * inv)[:, None])

    xb = x[core * BPC:(core + 1) * BPC]  # (4, 16, 12)
    put("xt", xb.transpose(2, 1, 0).reshape(12, 64))

    put("a0t", iv["a0_w"].T)
    put("a0b", iv["a0_b"][:, None])
    put("ait", iv["ai_w"].T)
    put("aib", iv["ai_b"][:, None])
    put("anT", np.concatenate([iv["an_w"][:, 16:80].T,
                               iv["an_w"][:, 0:16].T], axis=0))
    put("anb", iv["an_b"].reshape(2, 128).T)

    bs = (iv["b_ih"] + iv["b_hh"]).astype(np.float32)
    bs[512:] = iv["b_ih"][512:]
    put("bsgi", bs.reshape(6, 128).T)
    put("bhhn", np.repeat(iv["b_hh"][512:].reshape(2, 128).T, 4, axis=1))
    put("fnb", iv["fn_b"][:, None])
    put("brz4", bs[:512].reshape(4, 128))
    put("bin2", iv["b_ih"][512:].reshape(2, 128))
    put("bhhn2", iv["b_hh"][512:].reshape(2, 128))
    put("fib2", iv["fi_b"].reshape(2, 128))
    put("bsgi6", bs.reshape(6, 128))
    put("sel16", np.repeat(np.eye(4, dtype=np.float32), 4, axis=1))
    put("sel8", np.repeat(np.eye(2, dtype=np.float32), 4, axis=1))
    put("sel48", np.repeat(np.eye(6, dtype=np.float32), 8, axis=1))
    return smm, smb


def make_in_maps(inputs):
    iv = {k: np.asarray(v, np.float32) for k, v in inputs.items()}
    frames = iv["frames"]
    x = iv["x"]
    fr_all = _prep_frames(frames)
    w = _prep_weights(iv)
    in_maps = []
    for c in range(NCORES):
        smm, smb = _prep_smalls(iv, x, c)
        m = {"fr": np.ascontiguousarray(fr_all[c]), "smm": smm, "smb": smb}
        m.update(w)
        in_maps.append(m)
    return in_maps


_NC_CACHE = None


def get_nc():
    global _NC_CACHE
    if _NC_CACHE is None:
        _NC_CACHE = build_nc()
    return _NC_CACHE


def _install_ntff_hook():
    """The agent image's antenv lacks axon_hooks; synthesize it so
    run_bass_kernel_spmd(trace=True) can capture NTFF profiles."""
    try:
        from antenv.axon_hooks import get_axon_ntff_profile_hook  # noqa: F401
        return True
    except ImportError:
        pass
    try:
        import types
        import antenv
        if "/root/.axon_site" not in sys.path:
            sys.path.insert(0, "/root/.axon_site")
        from trn_agent_boot.trn_boot import _ntff_profile_via_ctypes
        hook = _ntff_profile_via_ctypes("/opt/axon/libaxon_pjrt.so")
        mod = types.ModuleType("antenv.axon_hooks")
        mod.get_axon_ntff_profile_hook = lambda: hook
        mod.set_axon_ntff_profile_hook = lambda h: None
        sys.modules["antenv.axon_hooks"] = mod
        antenv.axon_hooks = mod
        return hook is not None
    except Exception as e:  # pragma: no cover - profiling is best-effort
        print(f"ntff hook install failed: {e}")
        return False


def kernel(**inputs):
    global LAST_EXEC_NS, LAST_RESULTS
    nc = get_nc()
    in_maps = make_in_maps(inputs)
    trace = bool(int(os.environ.get("KERNEL_TRACE", "0")))
    if trace:
        trace = _install_ntff_hook()
    res = run_bass_kernel_spmd(nc, in_maps, core_ids=list(range(NCORES)),
                               trace=trace)
    LAST_RESULTS = res
    LAST_EXEC_NS = res.exec_time_ns
    outs = []
    for c in range(NCORES):
        o = res.results[c]["out"]  # (2, 40)
        outs.append(o.reshape(2, HOR, BPC).transpose(1, 2, 0)[:, :, None, :])
    return np.concatenate(outs, axis=1).astype(np.float32)


if __name__ == "__main__":
    nc = get_nc()
    print("built ok; instructions:",
          sum(len(bb.instructions) for bb in nc.main_func.blocks))
